# revision 2
# baseline (speedup 1.0000x reference)
"""ASPP pooling head on Trainium2 (Bass/Tile), data-parallel over batch on 8 cores.

Per sample: pooled = mean(x, spatial); y = relu((pooled @ W.T)*bn_scale + bn_shift);
out = broadcast(y, spatial).

Per core (2 samples): 64 MiB of x reads + 4 MiB bf16 output writes + 1 MiB weights,
DMA-bound at the ~435 GB/s SBUF-AXI fabric ceiling (steady-state trace: 414-433 GB/s).
Design, from trace analysis of the previous best (190.3 us):
  - x streamed as [128ch, 4096] f32 tiles on the sync HWDGE queue; first-sample
    chunks 1,3 on the scalar queue to shorten the ramp; wt/shift on the gpsimd
    (SWDGE) queue so the two HWDGE rings carry only x during ramp.
  - wt host-prearranged to [128, KCH*COUT] bf16 so its load is one contiguous DMA.
  - spatial sums produced DIRECTLY as bf16 (DVE tensor_reduce dst / ACT accum_out;
    both accumulate fp32 internally) - removes 32 pt-cast ACTIVATEs from the
    baseline's scalar queue and one 0.3us hop from the tail critical path.
  - 16 accumulating PE matmuls per (sample, o-block) with wt = W*bn_scale/4096
    folded on host.
  - broadcast/store: only a [128, 1024] bf16 piece is materialized per o-block
    (ScalarE Relu-activation from stride-0 PSUM src; DVE tensor_scalar for the
    last sample's second block); the store is ONE DMA per o-block whose source AP
    repeats the piece 4x (stride-0 mid-dim, 2 KB descriptors). This cuts the
    baseline's 8-piece bcast+store tail (~8 us of engine-serialized work) to
    ~2 us, and the 2x1.05 MB transfers drain on both HWDGE rings in parallel.
  - tail: the last chunk streams as 8 eighth-tiles with alternating ScalarE/DVE
    reduces (DVE last: no accumulator-read hop), so the final pooled value is
    ready ~0.7 us after the last x byte lands.
"""

import numpy as np

B, CIN, H, W_SP = 16, 2048, 64, 64
COUT = 256
NCORES = 8
BPC = B // NCORES
SP = H * W_SP
KCH = CIN // 128
NOB = COUT // 128
BN_EPS = 1e-5

SCA_CHUNKS = (1, 4, 7, 10, 13)  # chunks reduced on ScalarE
XIN_BUFS = 9
POOLED_BUFS = 24
BCAST_BUFS = 4
PSUM_BUFS = 4
PIECE = 1024                  # materialized bcast width; DMA repeats SP//PIECE x
TAIL_EIGHTHS = True           # last chunk as 8 pieces, alternating S/V (V last)

_CACHE = {}


def _build_nc():
    import concourse.bacc as bacc
    import concourse.mybir as mybir
    import concourse.tile as tile

    nc = bacc.Bacc("TRN2", target_bir_lowering=False, debug=False,
                   num_devices=NCORES)
    f32 = mybir.dt.float32
    bf16 = mybir.dt.bfloat16
    AT = mybir.ActivationFunctionType
    x = nc.dram_tensor("x", [BPC, CIN, SP], f32, kind="ExternalInput").ap()
    wt = nc.dram_tensor("wt", [128, KCH * COUT], bf16, kind="ExternalInput").ap()
    shift = nc.dram_tensor("shift", [COUT], f32, kind="ExternalInput").ap()
    out = nc.dram_tensor("out", [BPC, COUT, SP], bf16,
                         kind="ExternalOutput").ap()

    nrep = SP // PIECE

    with tile.TileContext(nc) as tc, \
         tc.tile_pool(name="consts", bufs=1) as consts, \
         tc.tile_pool(name="xin", bufs=XIN_BUFS) as xin, \
         tc.tile_pool(name="pooled", bufs=POOLED_BUFS) as pooledp, \
         tc.tile_pool(name="psum", bufs=PSUM_BUFS, space="PSUM") as psump, \
         tc.tile_pool(name="bcast", bufs=BCAST_BUFS) as bcastp:

        wt_sb = consts.tile([128, KCH * COUT], bf16)
        shift_sb = consts.tile([128, NOB], f32)
        nc.gpsimd.dma_start(wt_sb[:], wt)
        nc.gpsimd.dma_start(shift_sb[:], shift.rearrange("(ob p) -> p ob", p=128))
        zeros_col = consts.tile([128, 1], f32)
        nc.gpsimd.memset(zeros_col[:], 0.0)
        scratch = consts.tile([128, SP], f32)

        for b in range(BPC):
            last = b == BPC - 1
            pss = [psump.tile([128, 1], f32, name=f"ps{ob}", tag=f"ps{ob}")
                   for ob in range(NOB)]

            def reduce_and_mm(src_slice, width, k, scalar_eng, first, stop):
                xt = xin.tile([128, width], f32, name="xt", tag="xt")
                ramp_q = nc.scalar if (b == 0 and k in (1, 3)) else nc.sync
                ramp_q.dma_start(xt[:], src_slice)
                pt = pooledp.tile([128, 1], bf16, name="pt", tag="pt")
                with nc.allow_low_precision(reason="bf16 dst; fp32 internal accum"):
                    if scalar_eng:
                        nc.scalar.activation(scratch[:, :width], xt[:],
                                             AT.Identity, bias=zeros_col[:],
                                             scale=1.0, accum_out=pt[:])
                    else:
                        nc.vector.reduce_sum(pt[:], xt[:],
                                             axis=mybir.AxisListType.X)
                for ob in range(NOB):
                    nc.tensor.matmul(
                        pss[ob][:],
                        lhsT=wt_sb[:, k * COUT + ob * 128:
                                   k * COUT + ob * 128 + 128],
                        rhs=pt[:, 0:1],
                        start=first,
                        stop=stop,
                    )

            for k in range(KCH):
                src = x[b, k * 128:(k + 1) * 128, :]
                if TAIL_EIGHTHS and last and k == KCH - 1:
                    for d in range(8):
                        reduce_and_mm(src[:, d * (SP // 8):(d + 1) * (SP // 8)],
                                      SP // 8, k, d % 2 == 0,
                                      first=False, stop=(d == 7))
                    continue
                if (b == 0 and k == 0) or (last and k == KCH - 2):
                    for d in range(2):
                        sca = last and k == KCH - 2 and d == 0
                        reduce_and_mm(src[:, d * (SP // 2):(d + 1) * (SP // 2)],
                                      SP // 2, k, sca,
                                      first=(b == 0 and k == 0 and d == 0),
                                      stop=False)
                    continue
                reduce_and_mm(src, SP, k, k in SCA_CHUNKS,
                              first=(k == 0), stop=False)

            for ob in range(NOB):
                bc = bcastp.tile([128, PIECE], bf16, name=f"bc{ob}", tag="bc")
                src_b = pss[ob][:].broadcast_to([128, PIECE])
                if last and ob == 1:
                    nc.vector.tensor_scalar(
                        out=bc[:], in0=src_b,
                        scalar1=shift_sb[:, ob:ob + 1], scalar2=0.0,
                        op0=mybir.AluOpType.add, op1=mybir.AluOpType.max)
                    st_eng = nc.sync
                else:
                    nc.scalar.activation(bc[:], src_b, AT.Relu,
                                         bias=shift_sb[:, ob:ob + 1],
                                         scale=1.0)
                    st_eng = nc.scalar
                dst = out[b, ob * 128:(ob + 1) * 128, :].rearrange(
                    "c (r w) -> c r w", r=nrep)
                st_eng.dma_start(dst,
                                 bc[:].unsqueeze(1).broadcast_to(
                                     [128, nrep, PIECE]))

    nc.compile()
    return nc


def _prep_inputs(x, W, gamma, beta, running_mean, running_var):
    scale = np.asarray(gamma, np.float32) / np.sqrt(
        np.asarray(running_var, np.float32) + np.float32(BN_EPS))
    wt = np.ascontiguousarray(
        (np.asarray(W, np.float32) * scale[:, None]).T / np.float32(SP))
    wt_r = np.ascontiguousarray(
        wt.reshape(KCH, 128, COUT).transpose(1, 0, 2).reshape(128, KCH * COUT))
    import ml_dtypes
    wt_r = wt_r.astype(ml_dtypes.bfloat16)
    shift = (np.asarray(beta, np.float32)
             - np.asarray(running_mean, np.float32) * scale).astype(np.float32)
    xs = np.ascontiguousarray(np.asarray(x, np.float32)).reshape(
        NCORES, BPC, CIN, SP)
    return [{"x": xs[i], "wt": wt_r, "shift": shift} for i in range(NCORES)]


def kernel(x, W, gamma, beta, running_mean, running_var):
    from concourse import bass_utils

    if "nc" not in _CACHE:
        _CACHE["nc"] = _build_nc()
    nc = _CACHE["nc"]
    in_maps = _prep_inputs(x, W, gamma, beta, running_mean, running_var)
    res = bass_utils.run_bass_kernel_spmd(nc, in_maps,
                                          core_ids=list(range(NCORES)))
    outs = [np.asarray(res.results[i]["out"]).astype(np.float32)
            for i in range(NCORES)]
    return np.concatenate(outs, axis=0).reshape(B, COUT, H, W_SP)


# revision 4
# speedup vs baseline: 1.0356x; 1.0356x over previous
"""ASPP pooling head on Trainium2 (Bass/Tile), data-parallel over batch on 8 cores.

Per sample: pooled = mean(x, spatial); y = relu((pooled @ W.T)*bn_scale + bn_shift);
out = broadcast(y, spatial).

Per core (2 samples): 64 MiB of x reads + 4 MiB bf16 output writes + 1 MiB weights,
DMA-bound at the ~435 GB/s SBUF-AXI fabric ceiling (steady-state trace: 414-433 GB/s).
Design, from trace analysis of the previous best (190.3 us):
  - x streamed as [128ch, 4096] f32 tiles on the sync HWDGE queue; first-sample
    chunks 1,3 on the scalar queue to shorten the ramp; wt/shift on the gpsimd
    (SWDGE) queue so the two HWDGE rings carry only x during ramp.
  - wt host-prearranged to [128, KCH*COUT] bf16 so its load is one contiguous DMA.
  - spatial sums produced DIRECTLY as bf16 (DVE tensor_reduce dst / ACT accum_out;
    both accumulate fp32 internally) - removes 32 pt-cast ACTIVATEs from the
    baseline's scalar queue and one 0.3us hop from the tail critical path.
  - 16 accumulating PE matmuls per (sample, o-block) with wt = W*bn_scale/4096
    folded on host.
  - broadcast/store: only a [128, 1024] bf16 piece is materialized per o-block
    (ScalarE Relu-activation from stride-0 PSUM src; DVE tensor_scalar for the
    last sample's second block); the store is ONE DMA per o-block whose source AP
    repeats the piece 4x (stride-0 mid-dim, 2 KB descriptors). This cuts the
    baseline's 8-piece bcast+store tail (~8 us of engine-serialized work) to
    ~2 us, and the 2x1.05 MB transfers drain on both HWDGE rings in parallel.
  - tail: the last chunk streams as 8 eighth-tiles with alternating ScalarE/DVE
    reduces (DVE last: no accumulator-read hop), so the final pooled value is
    ready ~0.7 us after the last x byte lands.
"""

import numpy as np

B, CIN, H, W_SP = 16, 2048, 64, 64
COUT = 256
NCORES = 8
BPC = B // NCORES
SP = H * W_SP
KCH = CIN // 128
NOB = COUT // 128
BN_EPS = 1e-5

SCA_CHUNKS = (1, 4, 7, 10, 13)  # chunks reduced on ScalarE
XIN_BUFS = 9
POOLED_BUFS = 24
BCAST_BUFS = 4
PSUM_BUFS = 4
PIECE = 2048                  # materialized bcast width; DMA repeats SP//PIECE x
TAIL_EIGHTHS = True           # last chunk as 8 pieces, alternating S/V (V last)

_CACHE = {}


def _build_nc():
    import concourse.bacc as bacc
    import concourse.mybir as mybir
    import concourse.tile as tile

    nc = bacc.Bacc("TRN2", target_bir_lowering=False, debug=False,
                   num_devices=NCORES)
    f32 = mybir.dt.float32
    bf16 = mybir.dt.bfloat16
    AT = mybir.ActivationFunctionType
    x = nc.dram_tensor("x", [BPC, CIN, SP], f32, kind="ExternalInput").ap()
    wt = nc.dram_tensor("wt", [128, KCH * COUT], bf16, kind="ExternalInput").ap()
    shift = nc.dram_tensor("shift", [COUT], f32, kind="ExternalInput").ap()
    out = nc.dram_tensor("out", [BPC, COUT, SP], bf16,
                         kind="ExternalOutput").ap()

    nrep = SP // PIECE

    with tile.TileContext(nc) as tc, \
         tc.tile_pool(name="consts", bufs=1) as consts, \
         tc.tile_pool(name="xin", bufs=XIN_BUFS) as xin, \
         tc.tile_pool(name="pooled", bufs=POOLED_BUFS) as pooledp, \
         tc.tile_pool(name="psum", bufs=PSUM_BUFS, space="PSUM") as psump, \
         tc.tile_pool(name="bcast", bufs=BCAST_BUFS) as bcastp:

        wt_sb = consts.tile([128, KCH * COUT], bf16)
        shift_sb = consts.tile([128, NOB], f32)
        nc.scalar.dma_start(wt_sb[:], wt)
        nc.scalar.dma_start(shift_sb[:], shift.rearrange("(ob p) -> p ob", p=128))
        zeros_col = consts.tile([128, 1], f32)
        nc.gpsimd.memset(zeros_col[:], 0.0)
        scratch = consts.tile([128, SP], f32)

        for b in range(BPC):
            last = b == BPC - 1
            pss = [psump.tile([128, 1], f32, name=f"ps{ob}", tag=f"ps{ob}")
                   for ob in range(NOB)]

            def reduce_and_mm(src_slice, width, k, scalar_eng, first, stop):
                xt = xin.tile([128, width], f32, name="xt", tag="xt")
                ramp_q = nc.scalar if (b == 0 and k in (1, 3)) else nc.sync
                ramp_q.dma_start(xt[:], src_slice)
                pt = pooledp.tile([128, 1], bf16, name="pt", tag="pt")
                with nc.allow_low_precision(reason="bf16 dst; fp32 internal accum"):
                    if scalar_eng:
                        nc.scalar.activation(scratch[:, :width], xt[:],
                                             AT.Identity, bias=zeros_col[:],
                                             scale=1.0, accum_out=pt[:])
                    else:
                        nc.vector.reduce_sum(pt[:], xt[:],
                                             axis=mybir.AxisListType.X)
                for ob in range(NOB):
                    nc.tensor.matmul(
                        pss[ob][:],
                        lhsT=wt_sb[:, k * COUT + ob * 128:
                                   k * COUT + ob * 128 + 128],
                        rhs=pt[:, 0:1],
                        start=first,
                        stop=stop,
                    )

            for k in range(KCH):
                src = x[b, k * 128:(k + 1) * 128, :]
                if TAIL_EIGHTHS and last and k == KCH - 1:
                    for d in range(8):
                        reduce_and_mm(src[:, d * (SP // 8):(d + 1) * (SP // 8)],
                                      SP // 8, k, d % 2 == 0,
                                      first=False, stop=(d == 7))
                    continue
                if (b == 0 and k == 0) or (last and k == KCH - 2):
                    for d in range(2):
                        sca = last and k == KCH - 2 and d == 0
                        reduce_and_mm(src[:, d * (SP // 2):(d + 1) * (SP // 2)],
                                      SP // 2, k, sca,
                                      first=(b == 0 and k == 0 and d == 0),
                                      stop=False)
                    continue
                reduce_and_mm(src, SP, k, k in SCA_CHUNKS,
                              first=(k == 0), stop=False)

            for ob in range(NOB):
                bc = bcastp.tile([128, PIECE], bf16, name=f"bc{ob}", tag="bc")
                src_b = pss[ob][:].broadcast_to([128, PIECE])
                if last and ob == 1:
                    nc.vector.tensor_scalar(
                        out=bc[:], in0=src_b,
                        scalar1=shift_sb[:, ob:ob + 1], scalar2=0.0,
                        op0=mybir.AluOpType.add, op1=mybir.AluOpType.max)
                    st_eng = nc.sync
                else:
                    nc.scalar.activation(bc[:], src_b, AT.Relu,
                                         bias=shift_sb[:, ob:ob + 1],
                                         scale=1.0)
                    st_eng = nc.scalar
                dst = out[b, ob * 128:(ob + 1) * 128, :].rearrange(
                    "c (r w) -> c r w", r=nrep)
                st_eng.dma_start(dst,
                                 bc[:].unsqueeze(1).broadcast_to(
                                     [128, nrep, PIECE]))

    nc.compile()
    return nc


def _prep_inputs(x, W, gamma, beta, running_mean, running_var):
    scale = np.asarray(gamma, np.float32) / np.sqrt(
        np.asarray(running_var, np.float32) + np.float32(BN_EPS))
    wt = np.ascontiguousarray(
        (np.asarray(W, np.float32) * scale[:, None]).T / np.float32(SP))
    wt_r = np.ascontiguousarray(
        wt.reshape(KCH, 128, COUT).transpose(1, 0, 2).reshape(128, KCH * COUT))
    import ml_dtypes
    wt_r = wt_r.astype(ml_dtypes.bfloat16)
    shift = (np.asarray(beta, np.float32)
             - np.asarray(running_mean, np.float32) * scale).astype(np.float32)
    xs = np.ascontiguousarray(np.asarray(x, np.float32)).reshape(
        NCORES, BPC, CIN, SP)
    return [{"x": xs[i], "wt": wt_r, "shift": shift} for i in range(NCORES)]


def kernel(x, W, gamma, beta, running_mean, running_var):
    from concourse import bass_utils

    if "nc" not in _CACHE:
        _CACHE["nc"] = _build_nc()
    nc = _CACHE["nc"]
    in_maps = _prep_inputs(x, W, gamma, beta, running_mean, running_var)
    res = bass_utils.run_bass_kernel_spmd(nc, in_maps,
                                          core_ids=list(range(NCORES)))
    outs = [np.asarray(res.results[i]["out"]).astype(np.float32)
            for i in range(NCORES)]
    return np.concatenate(outs, axis=0).reshape(B, COUT, H, W_SP)
